# revision 10
# baseline (speedup 1.0000x reference)
"""Trainium2 Bass kernel for a cross-attention + adaLN-modulated-LN + linear block.

Sharding: 8 cores = 4 batches (B) x 2 token-halves of S=4096.  No collectives:
each core recomputes the (small) kv projection for its batch and processes all
16 attention heads for its 2048 tokens, then LN/modulation/final-linear for its
8 frames.  Host slices inputs per core and reassembles the output.

Device layout is feature-major ("transposed"): activations live as [C, tok]
tiles so every matmul contracts over the SBUF partition dim.  x and v are
transposed on the PE with an identity matmul.  Softmax runs on scoresT
[key, tok] tiles; the softmax denominator rides along the attention-output
matmul as an extra all-ones lhsT column.
"""

import sys

for _p in ("/opt/trn_rl_repo", "/opt/pypackages"):
    if _p not in sys.path:
        sys.path.append(_p)

import numpy as np

import concourse.bacc as bacc
import concourse.tile as tile
from concourse import mybir
from concourse.bass_utils import run_bass_kernel_spmd
from concourse.masks import make_identity

FP = mybir.dt.float32
AF = mybir.ActivationFunctionType
OP = mybir.AluOpType

# Problem sizes (hardcoded per spec).
B = 4
S = 4096
C = 1024
N2 = 512
H = 16
D = 64
T = 16
NT = 256          # tokens per frame
OUTD = 32

STOK = S // 2     # tokens per core
F = 8             # frames per core
G = C // 128      # 8 channel groups
TB = 512          # token block (matmul N)
NTB = STOK // TB  # 4
KB = N2 // 128    # 4 key blocks
SCALE = D ** -0.5
EPS = 1e-6
P = 128


def _body(nc, tc, io):
    x, v, tvec, cmat = io["x_sl"], io["v_b"], io["t_b"], io["c_sl"]
    wq, bq = io["wq"], io["bq"]
    wkv, bkv = io["wkv"], io["bkv"]
    wproj, bproj = io["wproj"], io["bproj"]
    wada, bada = io["wada"], io["bada"]
    wlin, blin = io["wlin"], io["blin"]
    yT = io["yT"]

    with (
        tc.tile_pool(name="consts", bufs=1) as consts,
        tc.tile_pool(name="xT", bufs=1) as xTp,
        tc.tile_pool(name="qa", bufs=1) as qap,
        tc.tile_pool(name="vv", bufs=1) as vvp,
        tc.tile_pool(name="kT", bufs=3) as kTp,
        tc.tile_pool(name="wp", bufs=4) as wp,
        tc.tile_pool(name="ps", bufs=8, space="PSUM") as psp,
    ):
        # ---- constants / small inputs ----
        ident = consts.tile([P, P], FP, tag="ident")
        make_identity(nc, ident)
        ones_t = consts.tile([P, P], FP, tag="ones")
        nc.vector.memset(ones_t, 1.0)
        eps_t = consts.tile([P, 1], FP, tag="eps")
        nc.vector.memset(eps_t, EPS)

        bq_t = consts.tile([P, G], FP, tag="bq")
        nc.sync.dma_start(out=bq_t, in_=bq.ap().rearrange("(g p) -> p g", p=P))
        bkvk_t = consts.tile([P, G], FP, tag="bkvk")
        nc.sync.dma_start(
            out=bkvk_t, in_=bkv.ap()[0:C].rearrange("(g p) -> p g", p=P)
        )
        bkvv_row = consts.tile([1, C], FP, tag="bkvvrow")
        nc.sync.dma_start(
            out=bkvv_row, in_=bkv.ap()[C : 2 * C].rearrange("(one n) -> one n", one=1)
        )
        bproj_t = consts.tile([P, G], FP, tag="bproj")
        nc.sync.dma_start(out=bproj_t, in_=bproj.ap().rearrange("(g p) -> p g", p=P))
        bada_t = consts.tile([P, 16], FP, tag="bada")
        nc.sync.dma_start(out=bada_t, in_=bada.ap().rearrange("(g p) -> p g", p=P))
        blin_t = consts.tile([OUTD, 1], FP, tag="blin")
        nc.sync.dma_start(
            out=blin_t, in_=blin.ap().rearrange("(o one) -> o one", one=1)
        )
        t_t = consts.tile([P, G], FP, tag="tvec")
        nc.sync.dma_start(out=t_t, in_=tvec.ap().rearrange("(g p) -> p g", p=P))

        # ---- persistent activation buffers ----
        # vv layout per head pair g (heads 2g, 2g+1), segment of 193 cols:
        #   [0:64]    even-head data   -> lhsT [.. : ..+65], denom row 64
        #   [64]      ones (even-head denominator column)
        #   [65]      ones (odd-head denominator column, window col 0)
        #   [66:129]  junk (stays 1.0)
        #   [129:193] odd-head data    -> lhsT 128-wide window, data rows 64..127
        xT = [xTp.tile([P, STOK], FP, name=f"xT{g}", tag=f"xT{g}") for g in range(G)]
        qa = [qap.tile([P, STOK], FP, name=f"qa{g}", tag=f"qa{g}") for g in range(G)]
        vv = [vvp.tile([P, 8 * 193], FP, name=f"vv{kb}", tag=f"vv{kb}") for kb in range(KB)]
        for kb in range(KB):
            nc.vector.memset(vv[kb], 1.0)

        with tc.tile_pool(name="vT", bufs=1) as vTp:
            vT = [vTp.tile([P, N2], FP, name=f"vT{g}", tag=f"vT{g}") for g in range(G)]

            with tc.tile_pool(name="ld", bufs=3) as ldp:
                # ---- transpose v: vT[g] = v[:, g*128:...].T ----
                for kt in range(KB):
                    v_nat = ldp.tile([P, C], FP, tag="ld")
                    nc.sync.dma_start(out=v_nat, in_=v[kt * P : (kt + 1) * P, :])
                    for g in range(G):
                        pt = psp.tile([P, P], FP, tag="ps")
                        nc.tensor.transpose(pt, v_nat[:, g * P : (g + 1) * P], ident)
                        nc.any.tensor_copy(
                            out=vT[g][:, kt * P : (kt + 1) * P], in_=pt
                        )

                # ---- transpose x: xT[g] = x[:, g*128:...].T ----
                for tt in range(STOK // P):
                    x_nat = ldp.tile([P, C], FP, tag="ld")
                    nc.sync.dma_start(out=x_nat, in_=x[tt * P : (tt + 1) * P, :])
                    for g in range(G):
                        pt = psp.tile([P, P], FP, tag="ps")
                        nc.tensor.transpose(pt, x_nat[:, g * P : (g + 1) * P], ident)
                        nc.any.tensor_copy(
                            out=xT[g][:, tt * P : (tt + 1) * P], in_=pt
                        )

            # ---- vv (key-major value matrix, 65 cols/head: 64 data + ones) ----
            with tc.tile_pool(name="w512", bufs=3) as w512:
                for half in range(2):
                    pss = [psp.tile([P, TB], FP, name="psv", tag="ps") for _ in range(KB)]
                    for ci in range(G):
                        wv = w512.tile([P, TB], FP, tag="w512")
                        nc.sync.dma_start(
                            out=wv,
                            in_=wkv[
                                ci * P : (ci + 1) * P,
                                C + half * TB : C + (half + 1) * TB,
                            ],
                        )
                        for kb in range(KB):
                            nc.tensor.matmul(
                                pss[kb],
                                lhsT=vT[ci][:, kb * P : (kb + 1) * P],
                                rhs=wv,
                                start=(ci == 0),
                                stop=False,
                            )
                    for kb in range(KB):
                        # bias row: += ones[m] * bkv_v[n]
                        nc.tensor.matmul(
                            pss[kb],
                            lhsT=ones_t[0:1, :],
                            rhs=bkvv_row[0:1, half * TB : (half + 1) * TB],
                            start=False,
                            stop=True,
                        )
                        vvr = vv[kb].rearrange("p (a r) -> p a r", r=193)
                        src = pss[kb].rearrange("p (a q j) -> p a q j", q=2, j=64)
                        gs = slice(half * 4, half * 4 + 4)
                        nc.vector.tensor_copy(
                            out=vvr[:, gs, 0:64], in_=src[:, :, 0, :]
                        )
                        nc.vector.tensor_copy(
                            out=vvr[:, gs, 129:193], in_=src[:, :, 1, :]
                        )

            # ---- q projection: qa[g] = (x @ wq).T slice + bq ----
            for g in range(G):
                pst = [psp.tile([P, TB], FP, name="psq", tag="ps") for _ in range(NTB)]
                for ci in range(G):
                    wt = wp.tile([P, P], FP, tag="w")
                    nc.sync.dma_start(
                        out=wt, in_=wq[ci * P : (ci + 1) * P, g * P : (g + 1) * P]
                    )
                    for tb in range(NTB):
                        nc.tensor.matmul(
                            pst[tb],
                            lhsT=wt,
                            rhs=xT[ci][:, tb * TB : (tb + 1) * TB],
                            start=(ci == 0),
                            stop=(ci == G - 1),
                        )
                for tb in range(NTB):
                    nc.vector.tensor_scalar_add(
                        qa[g][:, tb * TB : (tb + 1) * TB], pst[tb], bq_t[:, g : g + 1]
                    )

            # ---- adaLN: silu(t + c) @ wada + bada  (tiny; fills PE gaps) ----
            c_nat = consts.tile([F, C], FP, tag="cnat")
            nc.sync.dma_start(out=c_nat, in_=cmat[:, :])
            silu_t = consts.tile([P, G, F], FP, tag="silu")
            for g in range(G):
                pt = psp.tile([P, F], FP, tag="ps")
                nc.tensor.transpose(
                    pt, c_nat[:, g * P : (g + 1) * P], ident[0:F, 0:F]
                )
                nc.scalar.activation(
                    out=silu_t[:, g, :],
                    in_=pt,
                    func=AF.Silu,
                    bias=t_t[:, g : g + 1],
                    scale=1.0,
                )
            ada_t = consts.tile([P, 16, F], FP, tag="ada")
            for ct in range(16):
                pa = psp.tile([P, F], FP, tag="ps")
                for ci in range(G):
                    wt = wp.tile([P, P], FP, tag="w")
                    nc.sync.dma_start(
                        out=wt, in_=wada[ci * P : (ci + 1) * P, ct * P : (ct + 1) * P]
                    )
                    nc.tensor.matmul(
                        pa,
                        lhsT=wt,
                        rhs=silu_t[:, ci, :],
                        start=(ci == 0),
                        stop=(ci == G - 1),
                    )
                if ct < 8:
                    # ada cols 0..1023 = shift -> ct 0..7
                    nc.vector.tensor_scalar_add(
                        ada_t[:, ct, :], pa, bada_t[:, ct : ct + 1]
                    )
                else:
                    # ada cols 1024..2047 = sc -> ct 8..15 hold (1 + sc)
                    nc.vector.tensor_scalar(
                        ada_t[:, ct, :],
                        pa,
                        bada_t[:, ct : ct + 1],
                        1.0,
                        op0=OP.add,
                        op1=OP.add,
                    )

            # ---- attention, streamed per channel-group g (heads 2g, 2g+1) ----
            with (
                tc.tile_pool(name="exp", bufs=3) as expp,
                tc.tile_pool(name="bc", bufs=2) as bcp,
            ):
                for g in range(G):
                    # kT[g] = (v @ wkv[:, g*128:...]).T + bkv  (feature-major keys)
                    psk = psp.tile([P, N2], FP, tag="ps")
                    for ci in range(G):
                        wt = wp.tile([P, P], FP, tag="w")
                        nc.sync.dma_start(
                            out=wt,
                            in_=wkv[ci * P : (ci + 1) * P, g * P : (g + 1) * P],
                        )
                        nc.tensor.matmul(
                            psk,
                            lhsT=wt,
                            rhs=vT[ci],
                            start=(ci == 0),
                            stop=(ci == G - 1),
                        )
                    kt_g = kTp.tile([P, N2], FP, tag="kT")
                    nc.vector.tensor_scalar_add(kt_g, psk, bkvk_t[:, g : g + 1])

                    for half in range(2):
                        h = 2 * g + half
                        r0 = half * 64
                        dr = 64 - 64 * half  # denom row: 64 (even h), 0 (odd h)
                        for tb in range(NTB):
                            tbs = slice(tb * TB, (tb + 1) * TB)
                            if half == 0:
                                ao_ps = psp.tile([65, TB], FP, tag="ps")
                                ao_rows = slice(0, 64)
                                lhs_lo, lhs_hi = g * 193, g * 193 + 65
                            else:
                                ao_ps = psp.tile([P, TB], FP, tag="ps")
                                ao_rows = slice(64, 128)
                                lhs_lo, lhs_hi = g * 193 + 65, g * 193 + 193
                            for kb in range(KB):
                                sc_ps = psp.tile([P, TB], FP, tag="ps")
                                nc.tensor.matmul(
                                    sc_ps,
                                    lhsT=kt_g[r0 : r0 + 64, kb * P : (kb + 1) * P],
                                    rhs=qa[g][r0 : r0 + 64, tbs],
                                    start=True,
                                    stop=True,
                                )
                                ex = expp.tile([P, TB], FP, tag="e")
                                nc.scalar.activation(
                                    out=ex, in_=sc_ps, func=AF.Exp, scale=SCALE
                                )
                                nc.tensor.matmul(
                                    ao_ps,
                                    lhsT=vv[kb][:, lhs_lo:lhs_hi],
                                    rhs=ex,
                                    start=(kb == 0),
                                    stop=(kb == KB - 1),
                                )
                            # softmax normalization: divide by the ones-column row
                            dnb = bcp.tile([P, TB], FP, tag="bc")
                            nc.scalar.copy(
                                out=dnb[dr : dr + 1, :], in_=ao_ps[dr : dr + 1, :]
                            )
                            nc.vector.reciprocal(
                                out=dnb[dr : dr + 1, :], in_=dnb[dr : dr + 1, :]
                            )
                            bc_ps = psp.tile([P, TB], FP, tag="ps")
                            nc.tensor.matmul(
                                bc_ps[r0 : r0 + 64, :],
                                lhsT=ones_t[dr : dr + 1, 0:64],
                                rhs=dnb[dr : dr + 1, :],
                                start=True,
                                stop=True,
                            )
                            nc.scalar.copy(
                                out=dnb[r0 : r0 + 64, :], in_=bc_ps[r0 : r0 + 64, :]
                            )
                            nc.vector.tensor_mul(
                                qa[g][r0 : r0 + 64, tbs],
                                ao_ps[ao_rows, :],
                                dnb[r0 : r0 + 64, :],
                            )

        # ---- proj + residual: x1T = xT + (aoT.T @ wproj).T + bproj ----
        for g in range(G):
            pst = [psp.tile([P, TB], FP, name="psq", tag="ps") for _ in range(NTB)]
            for ci in range(G):
                wt = wp.tile([P, P], FP, tag="w")
                nc.sync.dma_start(
                    out=wt, in_=wproj[ci * P : (ci + 1) * P, g * P : (g + 1) * P]
                )
                for tb in range(NTB):
                    nc.tensor.matmul(
                        pst[tb],
                        lhsT=wt,
                        rhs=qa[ci][:, tb * TB : (tb + 1) * TB],
                        start=(ci == 0),
                        stop=(ci == G - 1),
                    )
            for tb in range(NTB):
                tbs = slice(tb * TB, (tb + 1) * TB)
                nc.vector.scalar_tensor_tensor(
                    out=xT[g][:, tbs],
                    in0=pst[tb],
                    scalar=bproj_t[:, g : g + 1],
                    in1=xT[g][:, tbs],
                    op0=OP.add,
                    op1=OP.add,
                )

        # ---- LayerNorm stats + modulation + final linear, per token block ----
        with (
            tc.tile_pool(name="tmp", bufs=4) as tmpp,
            tc.tile_pool(name="st", bufs=4) as stp,
            tc.tile_pool(name="yo", bufs=2) as yop,
        ):
            for tb in range(NTB):
                tbs = slice(tb * TB, (tb + 1) * TB)
                ln_a = psp.tile([1, TB], FP, tag="ps")
                ln_b = psp.tile([1, TB], FP, tag="ps")
                for g in range(G):
                    sqt = tmpp.tile([P, TB], FP, tag="tmp")
                    nc.vector.tensor_mul(sqt, xT[g][:, tbs], xT[g][:, tbs])
                    nc.tensor.matmul(
                        ln_a,
                        lhsT=ones_t[:, 0:1],
                        rhs=xT[g][:, tbs],
                        start=(g == 0),
                        stop=(g == G - 1),
                    )
                    nc.tensor.matmul(
                        ln_b,
                        lhsT=ones_t[:, 0:1],
                        rhs=sqt,
                        start=(g == 0),
                        stop=(g == G - 1),
                    )
                # stats: A = mu -> mu*rstd ; Bst = var -> rstd
                a_st = stp.tile([1, TB], FP, tag="st")
                b_st = stp.tile([1, TB], FP, tag="st")
                nc.vector.tensor_scalar_mul(a_st, ln_a, 1.0 / C)
                nc.vector.tensor_mul(b_st, a_st, a_st)
                nc.vector.scalar_tensor_tensor(
                    out=b_st,
                    in0=ln_b,
                    scalar=1.0 / C,
                    in1=b_st,
                    op0=OP.mult,
                    op1=OP.subtract,
                )
                nc.scalar.activation(
                    out=b_st, in_=b_st, func=AF.Sqrt, bias=eps_t[0:1, :], scale=1.0
                )
                nc.vector.reciprocal(out=b_st, in_=b_st)
                nc.vector.tensor_mul(a_st, a_st, b_st)  # mu * rstd
                # broadcast to 128 partitions via K=1 outer-product matmuls
                bc_r = psp.tile([P, TB], FP, tag="ps")
                nc.tensor.matmul(
                    bc_r, lhsT=ones_t[0:1, :], rhs=b_st, start=True, stop=True
                )
                bc_m = psp.tile([P, TB], FP, tag="ps")
                nc.tensor.matmul(
                    bc_m, lhsT=ones_t[0:1, :], rhs=a_st, start=True, stop=True
                )
                # modulate: x1 * rstd - mu*rstd, then *(1+sc) + shift per frame
                for g in range(G):
                    mt = tmpp.tile([P, TB], FP, tag="tmp")
                    nc.vector.scalar_tensor_tensor(
                        out=mt,
                        in0=xT[g][:, tbs],
                        scalar=1.0,
                        in1=bc_r,
                        op0=OP.mult,
                        op1=OP.mult,
                    )
                    nc.vector.tensor_tensor(mt, mt, bc_m, OP.subtract)
                    for f2 in range(2):
                        f = tb * 2 + f2
                        fs = slice(f * NT, (f + 1) * NT)
                        nc.vector.tensor_scalar(
                            xT[g][:, fs],
                            mt[:, f2 * NT : (f2 + 1) * NT],
                            ada_t[:, 8 + g, f : f + 1],
                            ada_t[:, g, f : f + 1],
                            op0=OP.mult,
                            op1=OP.add,
                        )
                # final linear -> yT[:, tbs]
                y_ps = psp.tile([OUTD, TB], FP, tag="ps")
                for ci in range(G):
                    wt = wp.tile([P, OUTD], FP, tag="w")
                    nc.sync.dma_start(out=wt, in_=wlin[ci * P : (ci + 1) * P, :])
                    nc.tensor.matmul(
                        y_ps,
                        lhsT=wt,
                        rhs=xT[ci][:, tbs],
                        start=(ci == 0),
                        stop=(ci == G - 1),
                    )
                yt_sb = yop.tile([OUTD, TB], FP, tag="y")
                nc.vector.tensor_scalar_add(yt_sb, y_ps, blin_t[:, 0:1])
                nc.sync.dma_start(out=yT[:, tbs], in_=yt_sb)


def build_nc():
    nc = bacc.Bacc("TRN2", target_bir_lowering=False, debug=False)
    io = {
        "x_sl": nc.dram_tensor("x_sl", [STOK, C], FP, kind="ExternalInput"),
        "v_b": nc.dram_tensor("v_b", [N2, C], FP, kind="ExternalInput"),
        "t_b": nc.dram_tensor("t_b", [C], FP, kind="ExternalInput"),
        "c_sl": nc.dram_tensor("c_sl", [F, C], FP, kind="ExternalInput"),
        "wq": nc.dram_tensor("wq", [C, C], FP, kind="ExternalInput"),
        "bq": nc.dram_tensor("bq", [C], FP, kind="ExternalInput"),
        "wkv": nc.dram_tensor("wkv", [C, 2 * C], FP, kind="ExternalInput"),
        "bkv": nc.dram_tensor("bkv", [2 * C], FP, kind="ExternalInput"),
        "wproj": nc.dram_tensor("wproj", [C, C], FP, kind="ExternalInput"),
        "bproj": nc.dram_tensor("bproj", [C], FP, kind="ExternalInput"),
        "wada": nc.dram_tensor("wada", [C, 2 * C], FP, kind="ExternalInput"),
        "bada": nc.dram_tensor("bada", [2 * C], FP, kind="ExternalInput"),
        "wlin": nc.dram_tensor("wlin", [C, OUTD], FP, kind="ExternalInput"),
        "blin": nc.dram_tensor("blin", [OUTD], FP, kind="ExternalInput"),
        "yT": nc.dram_tensor("yT", [OUTD, STOK], FP, kind="ExternalOutput"),
    }
    with tile.TileContext(nc) as tc:
        _body(nc, tc, io)
    nc.compile()
    return nc


_CACHE = {}


def _get_nc():
    if "nc" not in _CACHE:
        _CACHE["nc"] = build_nc()
    return _CACHE["nc"]


def make_in_maps(x, v, t, c, wq, bq, wkv, bkv, wproj, bproj, wada, bada, wlin, blin):
    f32 = lambda a: np.ascontiguousarray(np.asarray(a, dtype=np.float32))
    x, v, t, c = f32(x), f32(v), f32(t), f32(c)
    shared = {
        "wq": f32(wq),
        "bq": f32(bq),
        "wkv": f32(wkv),
        "bkv": f32(bkv),
        "wproj": f32(wproj),
        "bproj": f32(bproj),
        "wada": f32(wada),
        "bada": f32(bada),
        "wlin": f32(wlin),
        "blin": f32(blin),
    }
    in_maps = []
    for m in range(8):
        b, half = divmod(m, 2)
        in_maps.append(
            {
                "x_sl": f32(x[b, half * STOK : (half + 1) * STOK, :]),
                "v_b": f32(v[b]),
                "t_b": f32(t[b]),
                "c_sl": f32(c[b, half * F : (half + 1) * F, :]),
                **shared,
            }
        )
    return in_maps


def assemble_y(results):
    y = np.empty((B, T, NT, OUTD), np.float32)
    for m in range(8):
        b, half = divmod(m, 2)
        yt = np.asarray(results[m]["yT"])  # [OUTD, STOK]
        y[b, half * F : (half + 1) * F] = yt.T.reshape(F, NT, OUTD)
    return y


def kernel(x, v, t, c, wq, bq, wkv, bkv, wproj, bproj, wada, bada, wlin, blin, T=16, H=16):
    nc = _get_nc()
    in_maps = make_in_maps(
        x, v, t, c, wq, bq, wkv, bkv, wproj, bproj, wada, bada, wlin, blin
    )
    res = run_bass_kernel_spmd(nc, in_maps, core_ids=list(range(8)))
    return assemble_y(res.results)
